# revision 19
# baseline (speedup 1.0000x reference)
"""AttMaxPool2D (2x2 softmax-attention pooling) Trainium2 Bass kernel.

out[b, wo, ho, c] = sum_i p_i * exp(t*p_i) / sum_i exp(t*p_i)
over the 4 elements p_i of each 2x2 window of x[b, :, :, c] (softmax-
weighted pooling; jax.nn.softmax's max-subtraction cancels analytically).

Shipped kernel: _build2 ("v2", xbufs=3) — see its docstring. Design
drivers:
 - The graded number is a single cold execution. For the v1 layout the
   first execution of a fresh executable reproducibly costs ~520-630us
   MORE than the reps-delta steady state (measured via reps=65
   amplification: call2-warm ~ +40ms); that matches the graded
   754756ns vs v1's 165us steady state. v2 restructures every DMA to
   the descriptor floor — 32KB/descriptor, 1536 descriptors/rep in 16
   dma_starts (vs ~4350 in 50) — which ELIMINATES the cold penalty
   (call2-warm ~ 0 +/- 20us across fresh processes).
 - f32r matmuls (1 cyc/row vs 4 for fp32) let the PE absorb the h-pair
   reduction (PSUM accumulation of stride-2 slices) AND the w-pair
   contraction, dropping the DVE pre-sums entirely: DVE ~68us,
   PE ~60us, ACT ~55us, all under the 112us/core DMA floor (40MB at
   358GB/s). f32r keeps ~13 mantissa bits -> rel err 1.83e-4 vs the
   2e-2 gate.
 - Measured steady state (8-core SPMD, reps-delta): 112-118us/rep —
   at the HBM roofline. Full-image in-DMAs (v3) and in/out engine
   separation both measured slower; gpsimd can't read PSUM so the
   final mul stays on DVE.

v1 (_build) notes kept below for reference:

Layout (per core; batch-parallel across 8 cores, 4 images each):
 - SBUF tiles hold [w:128(partitions), (h_chunk:16, c:128)(free)] slabs of
   one image; the HBM read per partition is 8KB contiguous.
 - e = exp(t*x) on ScalarE, pe = x*e split across VectorE/GpSimd.
 - h-pair (window row) sums sE=e_even+e_odd, sP=pe_even+pe_odd run on
   VectorE/GpSimd with strided APs (full fp32).
 - w-pair (window col) contraction runs on the PE as fp32 matmuls against a
   pair-sum 0/1 weight matrix; two consecutive h-chunks write the two
   partition halves of one [128,F] PSUM tile (via 128-column weights whose
   nonzero block sits at [j*64, j*64+64)) so the finals run full-width.
 - r = 1/den via the fast custom-DVE reciprocal (~51 ULP), out = num*r on
   VectorE.

variant="f32r" instead feeds un-presummed e/pe straight to float32r
matmuls (PSUM-accumulating the h-pairs): ~2x faster PE and less vector
work, but the f32r data path keeps only ~13 mantissa bits (measured
~2e-4 rel err), so it is not the default.

Measured on HW (8-core SPMD, steady-state per-run via reps-delta):
  fp32 (default): ~121-140us median (fast-phase p25 ~61-87us), rel err 3.1e-6
  f32r:           ~117us, max rel err 1.8e-4
  pure-DMA floor: ~100us (40MB HBM traffic per core)
The shipped configuration uses fused hchunk=32 slabs (one DMA/exp/mul/
pair-sum op per 2MB slab, PSUM kept at [128,1024] granularity via the
partition-half weight packing): halving the per-op instruction overheads
measured ~20% faster than the hchunk=16 pipeline (149 vs 183us in a
drift-fair A/B; ~130us p25).
The kernel is VectorE-bound (10N elementwise ops: pe mul 4N, h-pair sums
4N, reciprocal+final 2N); GPSIMD offload was measured consistently
neutral-to-harmful and is disabled.
"""

import numpy as np
from contextlib import ExitStack

N_CORES = 8
B, W, H, C = 32, 128, 128, 128
BS = B // N_CORES            # images per core
HCHUNK = 16                  # h rows per slab
NPP = H // (2 * HCHUNK)      # psum iterations per image (h-chunk pairs)
WO, HO = W // 2, H // 2
FREE = HCHUNK * C            # slab free size (2048 f32)
PFREE = (HCHUNK // 2) * C    # psum free size (1024 f32)

# rows of each slab's pe-multiply done on GpSimd (of HCHUNK)
GP_PE_ROWS_F32 = 0
GP_PE_ROWS_F32R = 8


def _build(temp: float, reps: int = 1, variant: str = "fp32",
           dma_only: bool = False, no_pe: bool = False,
           gp_rows_ovr: int = None, direct_den: bool = False,
           hchunk: int = HCHUNK, sp_gp: bool = False,
           xbufs: int = 5, ebufs: int = 4, pebufs: int = 4, sbufs: int = 4,
           psbufs: int = None, out_alt: bool = False, robufs: int = 3,
           fused: bool = False, narrow_w: bool = False):
    import concourse.bacc as bacc
    import concourse.tile as tile
    from concourse import mybir

    f32 = mybir.dt.float32
    f32r = mybir.dt.float32r
    use_f32r = variant == "f32r"
    edt = f32r if use_f32r else f32

    free = hchunk * C
    pfree = (hchunk // 2) * C
    npp = H // (2 * hchunk)
    nq = pfree // 512
    if psbufs is None:
        psbufs = max(1, 8 // (2 * (pfree // 512 * 1)))
        psbufs = min(psbufs, 2)

    nc = bacc.Bacc("TRN2", target_bir_lowering=False, debug=False,
                   num_devices=N_CORES)
    x_ap = nc.dram_tensor("x", [BS, W, H, C], f32, kind="ExternalInput").ap()
    w_ap = nc.dram_tensor("wmat", [2, W, 128], edt,
                          kind="ExternalInput").ap()
    out_ap = nc.dram_tensor("out", [BS, WO, HO, C], f32,
                            kind="ExternalOutput").ap()

    with tile.TileContext(nc) as tc:
        with ExitStack() as ctx:
            wpool = ctx.enter_context(tc.tile_pool(name="w", bufs=1))
            xpool = ctx.enter_context(tc.tile_pool(name="x", bufs=xbufs))
            epool = ctx.enter_context(tc.tile_pool(name="e", bufs=ebufs))
            pepool = ctx.enter_context(tc.tile_pool(name="pe", bufs=pebufs))
            spool = ctx.enter_context(tc.tile_pool(name="s", bufs=sbufs))
            rpool = ctx.enter_context(tc.tile_pool(name="r", bufs=robufs))
            opool = ctx.enter_context(tc.tile_pool(name="o", bufs=robufs))
            pspool = ctx.enter_context(
                tc.tile_pool(name="ps", bufs=psbufs, space="PSUM"))

            wm = wpool.tile([W, 256], edt)
            nc.sync.dma_start(wm[:, 0:128], w_ap[0])
            nc.sync.dma_start(wm[:, 128:256], w_ap[1])

            gp_rows = GP_PE_ROWS_F32R if use_f32r else GP_PE_ROWS_F32
            if gp_rows_ovr is not None:
                gp_rows = gp_rows_ovr
            if fused:
                # hchunk=32 slabs, single big ops, [128,1024] psum granularity
                assert hchunk == 32 and variant == "fp32"
                for _rep in range(reps):
                    for b in range(BS):
                        for sl in range(4):
                            t3 = xpool.tile([128, free], f32, tag="t",
                                            name="t3").rearrange(
                                "p (h c) -> p h c", h=hchunk)
                            eng = nc.sync if (sl % 2 == 0) else nc.scalar
                            eng.dma_start(
                                t3,
                                x_ap[b, :, sl * hchunk:(sl + 1) * hchunk, :])
                            e3 = epool.tile([128, free], f32, tag="e",
                                            name="e3").rearrange(
                                "p (h c) -> p h c", h=hchunk)
                            nc.scalar.activation(
                                e3, t3, mybir.ActivationFunctionType.Exp,
                                scale=float(temp))
                            pe3 = pepool.tile([128, free], f32, tag="pe",
                                              name="pe3").rearrange(
                                "p (h c) -> p h c", h=hchunk)
                            nc.vector.tensor_mul(pe3, t3, e3)
                            sE = spool.tile([128, free // 2], f32, tag="sE",
                                            name="sE").rearrange(
                                "p (h c) -> p h c", h=hchunk // 2)
                            sP = spool.tile([128, free // 2], f32, tag="sP",
                                            name="sP").rearrange(
                                "p (h c) -> p h c", h=hchunk // 2)
                            nc.vector.tensor_add(
                                sE, e3[:, 0::2, :], e3[:, 1::2, :])
                            if sp_gp:
                                nc.gpsimd.tensor_add(
                                    sP, pe3[:, 0::2, :], pe3[:, 1::2, :])
                            else:
                                nc.vector.tensor_add(
                                    sP, pe3[:, 0::2, :], pe3[:, 1::2, :])
                            den_ps = pspool.tile([128, 1024], f32)
                            num_ps = pspool.tile([128, 1024], f32)
                            for g in range(2):
                                wm_g = wm[:, g * 128:(g + 1) * 128]
                                for q in range(2):
                                    h0 = g * 8 + q * 4
                                    if narrow_w:
                                        # [128,64] weight + dst partition
                                        # offset: halves each LDW (fp32 has
                                        # no fast-weight-load)
                                        ps_sl = (slice(g * 64, (g + 1) * 64),
                                                 slice(q * 512, (q + 1) * 512))
                                        nc.tensor.matmul(
                                            den_ps[ps_sl], wm[:, 0:64],
                                            sE[:, h0:h0 + 4, :],
                                            start=True, stop=True)
                                        nc.tensor.matmul(
                                            num_ps[ps_sl], wm[:, 0:64],
                                            sP[:, h0:h0 + 4, :],
                                            start=True, stop=True)
                                        continue
                                    ps_sl = (slice(0, 128),
                                             slice(q * 512, (q + 1) * 512))
                                    nc.tensor.matmul(
                                        den_ps[ps_sl], wm_g,
                                        sE[:, h0:h0 + 4, :],
                                        start=(g == 0), stop=(g == 1))
                                    nc.tensor.matmul(
                                        num_ps[ps_sl], wm_g,
                                        sP[:, h0:h0 + 4, :],
                                        start=(g == 0), stop=(g == 1))
                            r = rpool.tile([128, 1024], f32)
                            nc.vector.reciprocal_approx_fast(r[:], den_ps[:])
                            o = opool.tile([128, 1024], f32)
                            nc.vector.tensor_mul(o[:], num_ps[:], r[:])
                            o3 = o.rearrange("p (h c) -> p h c", h=8)
                            for g in range(2):
                                ho0 = sl * 16 + g * 8
                                nc.sync.dma_start(
                                    out_ap[b, :, ho0:ho0 + 8, :],
                                    o3[g * 64:(g + 1) * 64, :, :])
            for _rep in range(reps if not fused else 0):
                for b in range(BS):
                    for pp in range(npp):
                        den_ps = pspool.tile([128, pfree], f32)
                        num_ps = pspool.tile([128, pfree], f32)
                        for j2 in range(2):
                            hp = 2 * pp + j2
                            t3 = xpool.tile([128, free], f32, tag="t",
                                            name="t3").rearrange(
                                "p (h c) -> p h c", h=hchunk)
                            eng = nc.sync if (hp % 2 == 0) else nc.scalar
                            eng.dma_start(
                                t3,
                                x_ap[b, :, hp * hchunk:(hp + 1) * hchunk, :])
                            if dma_only:
                                continue
                            e3 = epool.tile([128, free], edt, tag="e",
                                            name="e3").rearrange(
                                "p (h c) -> p h c", h=hchunk)
                            nc.scalar.activation(
                                e3, t3, mybir.ActivationFunctionType.Exp,
                                scale=float(temp))
                            pe3 = pepool.tile([128, free], edt, tag="pe",
                                              name="pe3").rearrange(
                                "p (h c) -> p h c", h=hchunk)
                            k = hchunk - gp_rows
                            nc.vector.tensor_mul(
                                pe3[:, :k, :], t3[:, :k, :], e3[:, :k, :])
                            if gp_rows:
                                nc.gpsimd.tensor_mul(
                                    pe3[:, k:, :], t3[:, k:, :], e3[:, k:, :])
                            wm_j = wm[:, j2 * 128:(j2 + 1) * 128]
                            if use_f32r:
                                for q in range(nq):
                                    for dh in range(2):
                                        h0 = q * 8 + dh
                                        h1 = q * 8 + 8
                                        ps_sl = (slice(0, 128),
                                                 slice(q * 512,
                                                       (q + 1) * 512))
                                        st = (j2 == 0 and dh == 0)
                                        sp = (j2 == 1 and dh == 1)
                                        nc.tensor.matmul(
                                            den_ps[ps_sl], wm_j,
                                            e3[:, h0:h1:2, :],
                                            start=st, stop=sp)
                                        nc.tensor.matmul(
                                            num_ps[ps_sl], wm_j,
                                            pe3[:, h0:h1:2, :],
                                            start=st, stop=sp)
                            else:
                                sE = spool.tile([128, pfree], f32, tag="sE",
                                                name="sE").rearrange(
                                    "p (h c) -> p h c", h=hchunk // 2)
                                sP = spool.tile([128, pfree], f32, tag="sP",
                                                name="sP").rearrange(
                                    "p (h c) -> p h c", h=hchunk // 2)
                                if not direct_den:
                                    nc.vector.tensor_add(
                                        sE, e3[:, 0::2, :], e3[:, 1::2, :])
                                if sp_gp:
                                    nc.gpsimd.tensor_add(
                                        sP, pe3[:, 0::2, :], pe3[:, 1::2, :])
                                else:
                                    nc.vector.tensor_add(
                                        sP, pe3[:, 0::2, :], pe3[:, 1::2, :])
                                if no_pe:
                                    ho0 = hp * (hchunk // 2)
                                    nc.sync.dma_start(
                                        out_ap[b, :,
                                               ho0:ho0 + hchunk // 2, :],
                                        sE[0:64, :, :])
                                    continue
                                for q in range(nq):
                                    ps_sl = (slice(0, 128),
                                             slice(q * 512, (q + 1) * 512))
                                    q0, q1 = q * 4, (q + 1) * 4
                                    if direct_den:
                                        for dh in range(2):
                                            h0 = q * 8 + dh
                                            h1 = q * 8 + 8
                                            nc.tensor.matmul(
                                                den_ps[ps_sl], wm_j,
                                                e3[:, h0:h1:2, :],
                                                start=(j2 == 0 and dh == 0),
                                                stop=(j2 == 1 and dh == 1))
                                    else:
                                        nc.tensor.matmul(
                                            den_ps[ps_sl], wm_j,
                                            sE[:, q0:q1, :],
                                            start=(j2 == 0), stop=(j2 == 1))
                                    nc.tensor.matmul(
                                        num_ps[ps_sl], wm_j, sP[:, q0:q1, :],
                                        start=(j2 == 0), stop=(j2 == 1))
                        if no_pe:
                            continue
                        if dma_only:
                            for j2 in range(2):
                                ho0 = pp * hchunk + j2 * (hchunk // 2)
                                nc.sync.dma_start(
                                    out_ap[b, :, ho0:ho0 + hchunk // 2, :],
                                    t3[j2 * 64:(j2 + 1) * 64,
                                       0:hchunk // 2, :])
                            continue
                        r = rpool.tile([128, pfree], f32)
                        nc.vector.reciprocal_approx_fast(r[:], den_ps[:])
                        o = opool.tile([128, pfree], f32)
                        nc.vector.tensor_mul(o[:], num_ps[:], r[:])
                        o3 = o.rearrange("p (h c) -> p h c", h=hchunk // 2)
                        for j2 in range(2):
                            ho0 = pp * hchunk + j2 * (hchunk // 2)
                            oeng = nc.scalar if (out_alt and j2 == 1) else nc.sync
                            oeng.dma_start(
                                out_ap[b, :, ho0:ho0 + hchunk // 2, :],
                                o3[j2 * 64:(j2 + 1) * 64, :, :])
    nc.compile()
    return nc


def _build2(temp: float, reps: int = 1, wdma: bool = False,
            xbufs: int = 2, ebufs: int = 2, pebufs: int = 2,
            obufs: int = 2, rbufs: int = 3, psbufs: int = 2,
            dma_only: bool = False, dma_mode: str = "alt",
            gp_omul: bool = False, full_in: bool = False,
            presum: bool = False):
    """v2: descriptor-minimal layout.

    Per core (4 images), per half-image (64 H rows):
      - 1 in-DMA  [128p(W), 8192f] (32KB/partition, 1 desc/partition)
      - slabs of 32 H rows: e=exp(t*x) (ACT), pe=x*e (DVE), then f32r
        matmuls against a [128,64] pair-sum weight contract the W pairs
        and PSUM-accumulate the H pairs (2 matmuls, stride-2 h slices).
        All matmul output lands on partitions 0-63 ([64,1024] psum).
      - r=1/den (fast DVE recip), out=num*r written into a [64,4096]
        half-image buffer; 1 out-DMA (16KB/partition, 1 desc/partition).
    The 0/1 weight is built on-device (memset + 2 affine_selects), so x
    is the only DMA'd input. Descriptors/rep: 1024 in + 512 out vs
    ~4350 for the per-slab v1 layout; DMA instructions 16 vs 50.
    """
    import concourse.bacc as bacc
    import concourse.tile as tile
    from concourse import mybir

    f32 = mybir.dt.float32
    f32r = mybir.dt.float32r

    nc = bacc.Bacc("TRN2", target_bir_lowering=False, debug=False,
                   num_devices=N_CORES)
    x_ap = nc.dram_tensor("x", [BS, W, H, C], f32, kind="ExternalInput").ap()
    if wdma:
        w_ap = nc.dram_tensor("wmat", [W, 64], f32r,
                              kind="ExternalInput").ap()
    out_ap = nc.dram_tensor("out", [BS, WO, HO, C], f32,
                            kind="ExternalOutput").ap()

    with tile.TileContext(nc) as tc:
        with ExitStack() as ctx:
            wpool = ctx.enter_context(tc.tile_pool(name="w", bufs=1))
            xpool = ctx.enter_context(tc.tile_pool(name="x", bufs=xbufs))
            epool = ctx.enter_context(tc.tile_pool(name="e", bufs=ebufs))
            pepool = ctx.enter_context(tc.tile_pool(name="pe", bufs=pebufs))
            opool = ctx.enter_context(tc.tile_pool(name="o", bufs=obufs))
            rpool = ctx.enter_context(tc.tile_pool(name="r", bufs=rbufs))
            if presum:
                spool = ctx.enter_context(tc.tile_pool(name="s", bufs=2))
            pspool = ctx.enter_context(
                tc.tile_pool(name="ps", bufs=psbufs, space="PSUM"))

            wm = wpool.tile([W, 64], f32r)
            if wdma:
                nc.sync.dma_start(wm, w_ap)
            else:
                # wf[p, c] = 1.0 iff p//2 == c  <=>  0 <= p - 2c <= 1
                # (built in f32 — gpsimd memset/select reject f32r — then
                # bit-copied into the f32r matmul weight tile)
                wf = wpool.tile([W, 64], f32)
                nc.gpsimd.memset(wf[:], 1.0)
                nc.gpsimd.affine_select(
                    out=wf[:], in_=wf[:], pattern=[[-2, 64]],
                    compare_op=mybir.AluOpType.is_ge, fill=0.0,
                    base=0, channel_multiplier=1)
                nc.gpsimd.affine_select(
                    out=wf[:], in_=wf[:], pattern=[[2, 64]],
                    compare_op=mybir.AluOpType.is_ge, fill=0.0,
                    base=1, channel_multiplier=-1)
                nc.scalar.copy(wm[:], wf[:])

            slab_h = 16 if full_in else 32
            n_sl = 64 // slab_h
            for _rep in range(reps):
                for b in range(BS):
                    if full_in:
                        xfull = xpool.tile([128, 16384], f32, tag="x",
                                           name="ximg").rearrange(
                            "p (h c) -> p h c", h=128)
                        feng = nc.sync if b % 2 == 0 else nc.scalar
                        feng.dma_start(xfull, x_ap[b])
                    for hf in range(2):
                        if dma_mode == "alt":
                            ieng = nc.sync if hf == 0 else nc.scalar
                            oeng = nc.scalar if hf == 0 else nc.sync
                        else:  # "inout": all ins on SP, all outs on ACT
                            ieng, oeng = nc.sync, nc.scalar
                        if full_in:
                            ximg = xfull[:, hf * 64:(hf + 1) * 64, :]
                        else:
                            ximg = xpool.tile([128, 8192], f32, tag="x",
                                              name="ximg").rearrange(
                                "p (h c) -> p h c", h=64)
                            ieng.dma_start(
                                ximg, x_ap[b, :, hf * 64:(hf + 1) * 64, :])
                        if dma_only:
                            oeng.dma_start(
                                out_ap[b, :, hf * 32:(hf + 1) * 32, :],
                                ximg[0:64, 0:32, :])
                            continue
                        oimg = opool.tile([64, 4096], f32, tag="o",
                                          name="oimg")
                        for sl in range(n_sl):
                            xs = ximg[:, sl * slab_h:(sl + 1) * slab_h, :]
                            e3 = epool.tile([128, slab_h * C], f32r, tag="e",
                                            name="e3").rearrange(
                                "p (h c) -> p h c", h=slab_h)
                            nc.scalar.activation(
                                e3, xs, mybir.ActivationFunctionType.Exp,
                                scale=float(temp))
                            pe3 = pepool.tile([128, slab_h * C], f32r,
                                              tag="pe", name="pe3").rearrange(
                                "p (h c) -> p h c", h=slab_h)
                            nc.vector.tensor_mul(pe3, xs, e3)
                            if presum:
                                # h-pair sums on DVE: halves the PE matmul
                                # count (1 start/stop matmul per 512-chunk)
                                sE = spool.tile([128, slab_h * C // 2], f32r,
                                                tag="sE", name="sE").rearrange(
                                    "p (h c) -> p h c", h=slab_h // 2)
                                sP = spool.tile([128, slab_h * C // 2], f32r,
                                                tag="sP", name="sP").rearrange(
                                    "p (h c) -> p h c", h=slab_h // 2)
                                nc.vector.tensor_add(
                                    sE, e3[:, 0::2, :], e3[:, 1::2, :])
                                nc.vector.tensor_add(
                                    sP, pe3[:, 0::2, :], pe3[:, 1::2, :])
                            for q in range(slab_h // 16):
                                den = pspool.tile([64, 1024], f32)
                                num = pspool.tile([64, 1024], f32)
                                for qq in range(2):
                                    h0 = q * 16 + qq * 8
                                    psl = (slice(0, 64),
                                           slice(qq * 512, (qq + 1) * 512))
                                    if presum:
                                        hp = q * 8 + qq * 4
                                        nc.tensor.matmul(
                                            den[psl], wm,
                                            sE[:, hp:hp + 4, :],
                                            start=True, stop=True)
                                        nc.tensor.matmul(
                                            num[psl], wm,
                                            sP[:, hp:hp + 4, :],
                                            start=True, stop=True)
                                        continue
                                    for dh in range(2):
                                        nc.tensor.matmul(
                                            den[psl], wm,
                                            e3[:, h0 + dh:h0 + 8:2, :],
                                            start=(dh == 0), stop=(dh == 1))
                                        nc.tensor.matmul(
                                            num[psl], wm,
                                            pe3[:, h0 + dh:h0 + 8:2, :],
                                            start=(dh == 0), stop=(dh == 1))
                                r = rpool.tile([64, 1024], f32)
                                nc.vector.reciprocal_approx_fast(
                                    r[:], den[:])
                                off = (sl * slab_h // 2 + q * 8) * C
                                meng = nc.gpsimd if gp_omul else nc.vector
                                meng.tensor_mul(
                                    oimg[:, off:off + 1024], num[:], r[:])
                        oeng.dma_start(
                            out_ap[b, :, hf * 32:(hf + 1) * 32, :],
                            oimg.rearrange("p (h c) -> p h c", h=32))
    nc.compile()
    return nc


def _wmat2() -> np.ndarray:
    w = np.zeros((W, 64), dtype=np.float32)
    w[np.arange(W), np.arange(W) // 2] = 1.0
    return w


def _wmat() -> np.ndarray:
    w = np.zeros((2, W, 128), dtype=np.float32)
    for j in range(2):
        w[j, np.arange(W), j * 64 + np.arange(W) // 2] = 1.0
    return w


def kernel(x: np.ndarray, temperature: np.ndarray) -> np.ndarray:
    from concourse.bass_utils import run_bass_kernel_spmd

    x = np.ascontiguousarray(np.asarray(x, dtype=np.float32))
    temp = float(np.asarray(temperature, dtype=np.float32).reshape(-1)[0])
    nc = _build2(temp, reps=1, xbufs=3)
    shards = np.split(x, N_CORES, axis=0)
    in_maps = [{"x": s} for s in shards]
    last_exc = None
    for _attempt in range(2):
        try:
            res = run_bass_kernel_spmd(nc, in_maps,
                                       core_ids=list(range(N_CORES)))
            break
        except Exception as exc:  # one retry in case of a wedged device
            last_exc = exc
    else:
        raise last_exc
    out = np.concatenate([res.results[i]["out"] for i in range(N_CORES)],
                         axis=0)
    return out.astype(np.float32)



# revision 22
# speedup vs baseline: 1.2557x; 1.2557x over previous
"""AttMaxPool2D (2x2 softmax-attention pooling) Trainium2 Bass kernel.

out[b, wo, ho, c] = sum_i p_i * exp(t*p_i) / sum_i exp(t*p_i)
over the 4 elements p_i of each 2x2 window of x[b, :, :, c] (softmax-
weighted pooling; jax.nn.softmax's max-subtraction cancels analytically).

Shipped kernel: _build2 ("v2", xbufs=3) — see its docstring. Design
drivers:
 - The graded number is a single cold execution. For the v1 layout the
   first execution of a fresh executable reproducibly costs ~520-630us
   MORE than the reps-delta steady state (measured via reps=65
   amplification: call2-warm ~ +40ms); that matches the graded
   754756ns vs v1's 165us steady state. v2 restructures every DMA to
   the descriptor floor — 32KB/descriptor, 1536 descriptors/rep in 16
   dma_starts (vs ~4350 in 50) — which ELIMINATES the cold penalty
   (call2-warm ~ 0 +/- 20us across fresh processes).
 - f32r matmuls (1 cyc/row vs 4 for fp32) let the PE absorb the h-pair
   reduction (PSUM accumulation of stride-2 slices) AND the w-pair
   contraction, dropping the DVE pre-sums entirely: DVE ~68us,
   PE ~60us, ACT ~55us, all under the 112us/core DMA floor (40MB at
   358GB/s). f32r keeps ~13 mantissa bits -> rel err 1.83e-4 vs the
   2e-2 gate.
 - Measured steady state (8-core SPMD, reps-delta): 112-122us/rep —
   at the HBM roofline. Full-image in-DMAs (v3) and in/out engine
   separation both measured slower; gpsimd can't read PSUM so the
   final mul stays on DVE.
 - The presum variants (halve the matmul count via DVE h-pair sums)
   all silently lose the fast f32r PE path (observed fp32-level
   3.1e-6 error + ~4x PE time -> 147-158us steady) regardless of
   source AP signature; only the shipped direct form (stride-2
   sources from [128,4096] tiles, start/stop-split accumulation
   pairs) keeps it. Do not "simplify" to presums.

v1 (_build) notes kept below for reference:

Layout (per core; batch-parallel across 8 cores, 4 images each):
 - SBUF tiles hold [w:128(partitions), (h_chunk:16, c:128)(free)] slabs of
   one image; the HBM read per partition is 8KB contiguous.
 - e = exp(t*x) on ScalarE, pe = x*e split across VectorE/GpSimd.
 - h-pair (window row) sums sE=e_even+e_odd, sP=pe_even+pe_odd run on
   VectorE/GpSimd with strided APs (full fp32).
 - w-pair (window col) contraction runs on the PE as fp32 matmuls against a
   pair-sum 0/1 weight matrix; two consecutive h-chunks write the two
   partition halves of one [128,F] PSUM tile (via 128-column weights whose
   nonzero block sits at [j*64, j*64+64)) so the finals run full-width.
 - r = 1/den via the fast custom-DVE reciprocal (~51 ULP), out = num*r on
   VectorE.

variant="f32r" instead feeds un-presummed e/pe straight to float32r
matmuls (PSUM-accumulating the h-pairs): ~2x faster PE and less vector
work, but the f32r data path keeps only ~13 mantissa bits (measured
~2e-4 rel err), so it is not the default.

Measured on HW (8-core SPMD, steady-state per-run via reps-delta):
  fp32 (default): ~121-140us median (fast-phase p25 ~61-87us), rel err 3.1e-6
  f32r:           ~117us, max rel err 1.8e-4
  pure-DMA floor: ~100us (40MB HBM traffic per core)
The shipped configuration uses fused hchunk=32 slabs (one DMA/exp/mul/
pair-sum op per 2MB slab, PSUM kept at [128,1024] granularity via the
partition-half weight packing): halving the per-op instruction overheads
measured ~20% faster than the hchunk=16 pipeline (149 vs 183us in a
drift-fair A/B; ~130us p25).
The kernel is VectorE-bound (10N elementwise ops: pe mul 4N, h-pair sums
4N, reciprocal+final 2N); GPSIMD offload was measured consistently
neutral-to-harmful and is disabled.
"""

import numpy as np
from contextlib import ExitStack

N_CORES = 8
B, W, H, C = 32, 128, 128, 128
BS = B // N_CORES            # images per core
HCHUNK = 16                  # h rows per slab
NPP = H // (2 * HCHUNK)      # psum iterations per image (h-chunk pairs)
WO, HO = W // 2, H // 2
FREE = HCHUNK * C            # slab free size (2048 f32)
PFREE = (HCHUNK // 2) * C    # psum free size (1024 f32)

# rows of each slab's pe-multiply done on GpSimd (of HCHUNK)
GP_PE_ROWS_F32 = 0
GP_PE_ROWS_F32R = 8


def _build(temp: float, reps: int = 1, variant: str = "fp32",
           dma_only: bool = False, no_pe: bool = False,
           gp_rows_ovr: int = None, direct_den: bool = False,
           hchunk: int = HCHUNK, sp_gp: bool = False,
           xbufs: int = 5, ebufs: int = 4, pebufs: int = 4, sbufs: int = 4,
           psbufs: int = None, out_alt: bool = False, robufs: int = 3,
           fused: bool = False, narrow_w: bool = False):
    import concourse.bacc as bacc
    import concourse.tile as tile
    from concourse import mybir

    f32 = mybir.dt.float32
    f32r = mybir.dt.float32r
    use_f32r = variant == "f32r"
    edt = f32r if use_f32r else f32

    free = hchunk * C
    pfree = (hchunk // 2) * C
    npp = H // (2 * hchunk)
    nq = pfree // 512
    if psbufs is None:
        psbufs = max(1, 8 // (2 * (pfree // 512 * 1)))
        psbufs = min(psbufs, 2)

    nc = bacc.Bacc("TRN2", target_bir_lowering=False, debug=False,
                   num_devices=N_CORES)
    x_ap = nc.dram_tensor("x", [BS, W, H, C], f32, kind="ExternalInput").ap()
    w_ap = nc.dram_tensor("wmat", [2, W, 128], edt,
                          kind="ExternalInput").ap()
    out_ap = nc.dram_tensor("out", [BS, WO, HO, C], f32,
                            kind="ExternalOutput").ap()

    with tile.TileContext(nc) as tc:
        with ExitStack() as ctx:
            wpool = ctx.enter_context(tc.tile_pool(name="w", bufs=1))
            xpool = ctx.enter_context(tc.tile_pool(name="x", bufs=xbufs))
            epool = ctx.enter_context(tc.tile_pool(name="e", bufs=ebufs))
            pepool = ctx.enter_context(tc.tile_pool(name="pe", bufs=pebufs))
            spool = ctx.enter_context(tc.tile_pool(name="s", bufs=sbufs))
            rpool = ctx.enter_context(tc.tile_pool(name="r", bufs=robufs))
            opool = ctx.enter_context(tc.tile_pool(name="o", bufs=robufs))
            pspool = ctx.enter_context(
                tc.tile_pool(name="ps", bufs=psbufs, space="PSUM"))

            wm = wpool.tile([W, 256], edt)
            nc.sync.dma_start(wm[:, 0:128], w_ap[0])
            nc.sync.dma_start(wm[:, 128:256], w_ap[1])

            gp_rows = GP_PE_ROWS_F32R if use_f32r else GP_PE_ROWS_F32
            if gp_rows_ovr is not None:
                gp_rows = gp_rows_ovr
            if fused:
                # hchunk=32 slabs, single big ops, [128,1024] psum granularity
                assert hchunk == 32 and variant == "fp32"
                for _rep in range(reps):
                    for b in range(BS):
                        for sl in range(4):
                            t3 = xpool.tile([128, free], f32, tag="t",
                                            name="t3").rearrange(
                                "p (h c) -> p h c", h=hchunk)
                            eng = nc.sync if (sl % 2 == 0) else nc.scalar
                            eng.dma_start(
                                t3,
                                x_ap[b, :, sl * hchunk:(sl + 1) * hchunk, :])
                            e3 = epool.tile([128, free], f32, tag="e",
                                            name="e3").rearrange(
                                "p (h c) -> p h c", h=hchunk)
                            nc.scalar.activation(
                                e3, t3, mybir.ActivationFunctionType.Exp,
                                scale=float(temp))
                            pe3 = pepool.tile([128, free], f32, tag="pe",
                                              name="pe3").rearrange(
                                "p (h c) -> p h c", h=hchunk)
                            nc.vector.tensor_mul(pe3, t3, e3)
                            sE = spool.tile([128, free // 2], f32, tag="sE",
                                            name="sE").rearrange(
                                "p (h c) -> p h c", h=hchunk // 2)
                            sP = spool.tile([128, free // 2], f32, tag="sP",
                                            name="sP").rearrange(
                                "p (h c) -> p h c", h=hchunk // 2)
                            nc.vector.tensor_add(
                                sE, e3[:, 0::2, :], e3[:, 1::2, :])
                            if sp_gp:
                                nc.gpsimd.tensor_add(
                                    sP, pe3[:, 0::2, :], pe3[:, 1::2, :])
                            else:
                                nc.vector.tensor_add(
                                    sP, pe3[:, 0::2, :], pe3[:, 1::2, :])
                            den_ps = pspool.tile([128, 1024], f32)
                            num_ps = pspool.tile([128, 1024], f32)
                            for g in range(2):
                                wm_g = wm[:, g * 128:(g + 1) * 128]
                                for q in range(2):
                                    h0 = g * 8 + q * 4
                                    if narrow_w:
                                        # [128,64] weight + dst partition
                                        # offset: halves each LDW (fp32 has
                                        # no fast-weight-load)
                                        ps_sl = (slice(g * 64, (g + 1) * 64),
                                                 slice(q * 512, (q + 1) * 512))
                                        nc.tensor.matmul(
                                            den_ps[ps_sl], wm[:, 0:64],
                                            sE[:, h0:h0 + 4, :],
                                            start=True, stop=True)
                                        nc.tensor.matmul(
                                            num_ps[ps_sl], wm[:, 0:64],
                                            sP[:, h0:h0 + 4, :],
                                            start=True, stop=True)
                                        continue
                                    ps_sl = (slice(0, 128),
                                             slice(q * 512, (q + 1) * 512))
                                    nc.tensor.matmul(
                                        den_ps[ps_sl], wm_g,
                                        sE[:, h0:h0 + 4, :],
                                        start=(g == 0), stop=(g == 1))
                                    nc.tensor.matmul(
                                        num_ps[ps_sl], wm_g,
                                        sP[:, h0:h0 + 4, :],
                                        start=(g == 0), stop=(g == 1))
                            r = rpool.tile([128, 1024], f32)
                            nc.vector.reciprocal_approx_fast(r[:], den_ps[:])
                            o = opool.tile([128, 1024], f32)
                            nc.vector.tensor_mul(o[:], num_ps[:], r[:])
                            o3 = o.rearrange("p (h c) -> p h c", h=8)
                            for g in range(2):
                                ho0 = sl * 16 + g * 8
                                nc.sync.dma_start(
                                    out_ap[b, :, ho0:ho0 + 8, :],
                                    o3[g * 64:(g + 1) * 64, :, :])
            for _rep in range(reps if not fused else 0):
                for b in range(BS):
                    for pp in range(npp):
                        den_ps = pspool.tile([128, pfree], f32)
                        num_ps = pspool.tile([128, pfree], f32)
                        for j2 in range(2):
                            hp = 2 * pp + j2
                            t3 = xpool.tile([128, free], f32, tag="t",
                                            name="t3").rearrange(
                                "p (h c) -> p h c", h=hchunk)
                            eng = nc.sync if (hp % 2 == 0) else nc.scalar
                            eng.dma_start(
                                t3,
                                x_ap[b, :, hp * hchunk:(hp + 1) * hchunk, :])
                            if dma_only:
                                continue
                            e3 = epool.tile([128, free], edt, tag="e",
                                            name="e3").rearrange(
                                "p (h c) -> p h c", h=hchunk)
                            nc.scalar.activation(
                                e3, t3, mybir.ActivationFunctionType.Exp,
                                scale=float(temp))
                            pe3 = pepool.tile([128, free], edt, tag="pe",
                                              name="pe3").rearrange(
                                "p (h c) -> p h c", h=hchunk)
                            k = hchunk - gp_rows
                            nc.vector.tensor_mul(
                                pe3[:, :k, :], t3[:, :k, :], e3[:, :k, :])
                            if gp_rows:
                                nc.gpsimd.tensor_mul(
                                    pe3[:, k:, :], t3[:, k:, :], e3[:, k:, :])
                            wm_j = wm[:, j2 * 128:(j2 + 1) * 128]
                            if use_f32r:
                                for q in range(nq):
                                    for dh in range(2):
                                        h0 = q * 8 + dh
                                        h1 = q * 8 + 8
                                        ps_sl = (slice(0, 128),
                                                 slice(q * 512,
                                                       (q + 1) * 512))
                                        st = (j2 == 0 and dh == 0)
                                        sp = (j2 == 1 and dh == 1)
                                        nc.tensor.matmul(
                                            den_ps[ps_sl], wm_j,
                                            e3[:, h0:h1:2, :],
                                            start=st, stop=sp)
                                        nc.tensor.matmul(
                                            num_ps[ps_sl], wm_j,
                                            pe3[:, h0:h1:2, :],
                                            start=st, stop=sp)
                            else:
                                sE = spool.tile([128, pfree], f32, tag="sE",
                                                name="sE").rearrange(
                                    "p (h c) -> p h c", h=hchunk // 2)
                                sP = spool.tile([128, pfree], f32, tag="sP",
                                                name="sP").rearrange(
                                    "p (h c) -> p h c", h=hchunk // 2)
                                if not direct_den:
                                    nc.vector.tensor_add(
                                        sE, e3[:, 0::2, :], e3[:, 1::2, :])
                                if sp_gp:
                                    nc.gpsimd.tensor_add(
                                        sP, pe3[:, 0::2, :], pe3[:, 1::2, :])
                                else:
                                    nc.vector.tensor_add(
                                        sP, pe3[:, 0::2, :], pe3[:, 1::2, :])
                                if no_pe:
                                    ho0 = hp * (hchunk // 2)
                                    nc.sync.dma_start(
                                        out_ap[b, :,
                                               ho0:ho0 + hchunk // 2, :],
                                        sE[0:64, :, :])
                                    continue
                                for q in range(nq):
                                    ps_sl = (slice(0, 128),
                                             slice(q * 512, (q + 1) * 512))
                                    q0, q1 = q * 4, (q + 1) * 4
                                    if direct_den:
                                        for dh in range(2):
                                            h0 = q * 8 + dh
                                            h1 = q * 8 + 8
                                            nc.tensor.matmul(
                                                den_ps[ps_sl], wm_j,
                                                e3[:, h0:h1:2, :],
                                                start=(j2 == 0 and dh == 0),
                                                stop=(j2 == 1 and dh == 1))
                                    else:
                                        nc.tensor.matmul(
                                            den_ps[ps_sl], wm_j,
                                            sE[:, q0:q1, :],
                                            start=(j2 == 0), stop=(j2 == 1))
                                    nc.tensor.matmul(
                                        num_ps[ps_sl], wm_j, sP[:, q0:q1, :],
                                        start=(j2 == 0), stop=(j2 == 1))
                        if no_pe:
                            continue
                        if dma_only:
                            for j2 in range(2):
                                ho0 = pp * hchunk + j2 * (hchunk // 2)
                                nc.sync.dma_start(
                                    out_ap[b, :, ho0:ho0 + hchunk // 2, :],
                                    t3[j2 * 64:(j2 + 1) * 64,
                                       0:hchunk // 2, :])
                            continue
                        r = rpool.tile([128, pfree], f32)
                        nc.vector.reciprocal_approx_fast(r[:], den_ps[:])
                        o = opool.tile([128, pfree], f32)
                        nc.vector.tensor_mul(o[:], num_ps[:], r[:])
                        o3 = o.rearrange("p (h c) -> p h c", h=hchunk // 2)
                        for j2 in range(2):
                            ho0 = pp * hchunk + j2 * (hchunk // 2)
                            oeng = nc.scalar if (out_alt and j2 == 1) else nc.sync
                            oeng.dma_start(
                                out_ap[b, :, ho0:ho0 + hchunk // 2, :],
                                o3[j2 * 64:(j2 + 1) * 64, :, :])
    nc.compile()
    return nc


def _build2(temp: float, reps: int = 1, wdma: bool = False,
            xbufs: int = 2, ebufs: int = 2, pebufs: int = 2,
            obufs: int = 2, rbufs: int = 3, psbufs: int = 2,
            dma_only: bool = False, dma_mode: str = "alt",
            gp_omul: bool = False, full_in: bool = False,
            presum: bool = False):
    """v2: descriptor-minimal layout.

    Per core (4 images), per half-image (64 H rows):
      - 1 in-DMA  [128p(W), 8192f] (32KB/partition, 1 desc/partition)
      - slabs of 32 H rows: e=exp(t*x) (ACT), pe=x*e (DVE), then f32r
        matmuls against a [128,64] pair-sum weight contract the W pairs
        and PSUM-accumulate the H pairs (2 matmuls, stride-2 h slices).
        All matmul output lands on partitions 0-63 ([64,1024] psum).
      - r=1/den (fast DVE recip), out=num*r written into a [64,4096]
        half-image buffer; 1 out-DMA (16KB/partition, 1 desc/partition).
    The 0/1 weight is built on-device (memset + 2 affine_selects), so x
    is the only DMA'd input. Descriptors/rep: 1024 in + 512 out vs
    ~4350 for the per-slab v1 layout; DMA instructions 16 vs 50.
    """
    import concourse.bacc as bacc
    import concourse.tile as tile
    from concourse import mybir

    f32 = mybir.dt.float32
    f32r = mybir.dt.float32r

    nc = bacc.Bacc("TRN2", target_bir_lowering=False, debug=False,
                   num_devices=N_CORES)
    x_ap = nc.dram_tensor("x", [BS, W, H, C], f32, kind="ExternalInput").ap()
    if wdma:
        w_ap = nc.dram_tensor("wmat", [W, 64], f32r,
                              kind="ExternalInput").ap()
    out_ap = nc.dram_tensor("out", [BS, WO, HO, C], f32,
                            kind="ExternalOutput").ap()

    with tile.TileContext(nc) as tc:
        with ExitStack() as ctx:
            wpool = ctx.enter_context(tc.tile_pool(name="w", bufs=1))
            xpool = ctx.enter_context(tc.tile_pool(name="x", bufs=xbufs))
            epool = ctx.enter_context(tc.tile_pool(name="e", bufs=ebufs))
            pepool = ctx.enter_context(tc.tile_pool(name="pe", bufs=pebufs))
            opool = ctx.enter_context(tc.tile_pool(name="o", bufs=obufs))
            rpool = ctx.enter_context(tc.tile_pool(name="r", bufs=rbufs))
            if presum:
                spool = ctx.enter_context(tc.tile_pool(name="s", bufs=2))
            pspool = ctx.enter_context(
                tc.tile_pool(name="ps", bufs=psbufs, space="PSUM"))

            wm = wpool.tile([W, 64], f32r)
            if wdma:
                nc.sync.dma_start(wm, w_ap)
            else:
                # wf[p, c] = 1.0 iff p//2 == c  <=>  0 <= p - 2c <= 1
                # (built in f32 — gpsimd memset/select reject f32r — then
                # bit-copied into the f32r matmul weight tile)
                wf = wpool.tile([W, 64], f32)
                nc.gpsimd.memset(wf[:], 1.0)
                nc.gpsimd.affine_select(
                    out=wf[:], in_=wf[:], pattern=[[-2, 64]],
                    compare_op=mybir.AluOpType.is_ge, fill=0.0,
                    base=0, channel_multiplier=1)
                nc.gpsimd.affine_select(
                    out=wf[:], in_=wf[:], pattern=[[2, 64]],
                    compare_op=mybir.AluOpType.is_ge, fill=0.0,
                    base=1, channel_multiplier=-1)
                nc.scalar.copy(wm[:], wf[:])

            slab_h = 16 if full_in else 32
            n_sl = 64 // slab_h
            for _rep in range(reps):
                for b in range(BS):
                    if full_in:
                        xfull = xpool.tile([128, 16384], f32, tag="x",
                                           name="ximg").rearrange(
                            "p (h c) -> p h c", h=128)
                        feng = nc.sync if b % 2 == 0 else nc.scalar
                        feng.dma_start(xfull, x_ap[b])
                    for hf in range(2):
                        if dma_mode == "alt":
                            ieng = nc.sync if hf == 0 else nc.scalar
                            oeng = nc.scalar if hf == 0 else nc.sync
                        else:  # "inout": all ins on SP, all outs on ACT
                            ieng, oeng = nc.sync, nc.scalar
                        if full_in:
                            ximg = xfull[:, hf * 64:(hf + 1) * 64, :]
                        else:
                            ximg = xpool.tile([128, 8192], f32, tag="x",
                                              name="ximg").rearrange(
                                "p (h c) -> p h c", h=64)
                            ieng.dma_start(
                                ximg, x_ap[b, :, hf * 64:(hf + 1) * 64, :])
                        if dma_only:
                            oeng.dma_start(
                                out_ap[b, :, hf * 32:(hf + 1) * 32, :],
                                ximg[0:64, 0:32, :])
                            continue
                        oimg = opool.tile([64, 4096], f32, tag="o",
                                          name="oimg")
                        for sl in range(n_sl):
                            xs = ximg[:, sl * slab_h:(sl + 1) * slab_h, :]
                            e3 = epool.tile([128, slab_h * C], f32r, tag="e",
                                            name="e3").rearrange(
                                "p (h c) -> p h c", h=slab_h)
                            nc.scalar.activation(
                                e3, xs, mybir.ActivationFunctionType.Exp,
                                scale=float(temp))
                            pe3 = pepool.tile([128, slab_h * C], f32r,
                                              tag="pe", name="pe3").rearrange(
                                "p (h c) -> p h c", h=slab_h)
                            nc.vector.tensor_mul(pe3, xs, e3)
                            if presum:
                                # h-pair sums on DVE: halves the PE matmul
                                # count (1 start/stop matmul per 512-chunk).
                                # sE/sP interleave even/odd h-slots of ONE
                                # [128, slab_h*C] tile so the matmul source
                                # APs keep the exact [[4096,128],[256,4],
                                # [1,128]] signature of the direct variant —
                                # narrower/contiguous sources make walrus
                                # drop the fast f32r PE path (observed:
                                # fp32-level error + 4x PE time).
                                sp = spool.tile([128, slab_h * C], f32r,
                                                tag="sp", name="sp").rearrange(
                                    "p (h c) -> p h c", h=slab_h)
                                nc.vector.tensor_add(
                                    sp[:, 0::2, :], e3[:, 0::2, :],
                                    e3[:, 1::2, :])
                                nc.vector.tensor_add(
                                    sp[:, 1::2, :], pe3[:, 0::2, :],
                                    pe3[:, 1::2, :])
                            for q in range(slab_h // 16):
                                den = pspool.tile([64, 1024], f32)
                                num = pspool.tile([64, 1024], f32)
                                for qq in range(2):
                                    h0 = q * 16 + qq * 8
                                    psl = (slice(0, 64),
                                           slice(qq * 512, (qq + 1) * 512))
                                    if presum:
                                        s0 = 2 * (q * 8 + qq * 4)
                                        nc.tensor.matmul(
                                            den[psl], wm,
                                            sp[:, s0:s0 + 8:2, :],
                                            start=True, stop=True)
                                        nc.tensor.matmul(
                                            num[psl], wm,
                                            sp[:, s0 + 1:s0 + 8:2, :],
                                            start=True, stop=True)
                                        continue
                                    for dh in range(2):
                                        nc.tensor.matmul(
                                            den[psl], wm,
                                            e3[:, h0 + dh:h0 + 8:2, :],
                                            start=(dh == 0), stop=(dh == 1))
                                        nc.tensor.matmul(
                                            num[psl], wm,
                                            pe3[:, h0 + dh:h0 + 8:2, :],
                                            start=(dh == 0), stop=(dh == 1))
                                r = rpool.tile([64, 1024], f32)
                                nc.vector.reciprocal_approx_fast(
                                    r[:], den[:])
                                off = (sl * slab_h // 2 + q * 8) * C
                                meng = nc.gpsimd if gp_omul else nc.vector
                                meng.tensor_mul(
                                    oimg[:, off:off + 1024], num[:], r[:])
                        oeng.dma_start(
                            out_ap[b, :, hf * 32:(hf + 1) * 32, :],
                            oimg.rearrange("p (h c) -> p h c", h=32))
    nc.compile()
    return nc


def _wmat2() -> np.ndarray:
    w = np.zeros((W, 64), dtype=np.float32)
    w[np.arange(W), np.arange(W) // 2] = 1.0
    return w


def _wmat() -> np.ndarray:
    w = np.zeros((2, W, 128), dtype=np.float32)
    for j in range(2):
        w[j, np.arange(W), j * 64 + np.arange(W) // 2] = 1.0
    return w


def kernel(x: np.ndarray, temperature: np.ndarray) -> np.ndarray:
    from concourse.bass_utils import run_bass_kernel_spmd

    x = np.ascontiguousarray(np.asarray(x, dtype=np.float32))
    temp = float(np.asarray(temperature, dtype=np.float32).reshape(-1)[0])
    nc = _build2(temp, reps=1, xbufs=3)
    shards = np.split(x, N_CORES, axis=0)
    in_maps = [{"x": s} for s in shards]
    last_exc = None
    for _attempt in range(2):
        try:
            res = run_bass_kernel_spmd(nc, in_maps,
                                       core_ids=list(range(N_CORES)))
            break
        except Exception as exc:  # one retry in case of a wedged device
            last_exc = exc
    else:
        raise last_exc
    out = np.concatenate([res.results[i]["out"] for i in range(N_CORES)],
                         axis=0)
    return out.astype(np.float32)

